# revision 14
# baseline (speedup 1.0000x reference)
"""Trainium2 Bass kernel for nn_CompatibleLearningLoss (MoCo-style queue contrastive loss).

Strategy: shard the queue dimension (Q=32768) across 8 NeuronCores (4096 rows
each).  Each core computes its slice of the three matmuls
    old_embeds  @ feat_queue_shard.T   -> weight
    new_e       @ feat_queue_shard.T   -> scores1
    new_logits  @ logit_queue_shard.T  -> scores2
and reduces per-row partial softmax statistics (chunk max, exp-sum vs chunk
max, masked-weighted raw sums) into a [128, 128] stats tile.  The host combines
the partials in float64 and produces the two scalar losses.

The kernel is HBM-bandwidth bound (the logit-queue stream dominates), so both
queue matrices travel as fp8e4 (TRN E4M3) and every matmul runs in
perf_mode=DoubleRow (K=256 per pass).  The moving operands are pre-scaled by
16 host-side to clear the e4m3 subnormal band; the 1/16 descale is folded into
the Exp activation / the maskw fuse / the host-side combine.  Measured loss
error ~3e-3 relative (gate is 2e-2).

Marshaling (all host-side): circular queue scatter, new_embeds normalization,
label mask, fp8 quantization, and pre-transposition of everything into its
SBUF image [partition, k-chunk, free] so each DMA is one contiguous run per
partition.
"""

from contextlib import nullcontext

import numpy as np

import concourse.bass as bass
import concourse.tile as tile
from concourse import mybir
from concourse.bass_utils import run_bass_kernel_spmd
from concourse.vector_clock import ScopedClock

N = 128      # batch
D = 512      # embed dim
C = 8192     # logit dim
Q = 32768    # queue length
N_CORES = 8
QS = Q // N_CORES          # 4096 queue rows per core
EPS = 1e-12
MSCALE = 16.0              # host pre-scale on the moving (queue) operands
INV = 1.0 / MSCALE

F32 = mybir.dt.float32
FP8 = mybir.dt.float8e4

# stats tile column layout (per core, [128, 128] f32)
# cols 0:8      m1 parts   (feat-path chunk maxes TIMES MSCALE, 8 chunks of 512)
# cols 8:16     z1 parts   (feat-path sum exp(s - chunk max))
# cols 16:24    a1 parts   (feat-path sum maskw * s_raw)
# cols 24:32    W  parts   (sum maskw)
# cols 32:32+B  m2 parts   (logit-path chunk maxes TIMES MSCALE, B_CHUNKS chunks of 512)
# cols 64:64+B  z2 parts
# cols 96:96+B  a2 parts
A_CHUNKS = 8
B_CHUNKS = 8


def _split_excess_waits(nc: bass.Bass, limit: int = 1) -> None:
    """This walrus build rejects instructions carrying more than one sync wait
    ("Too many sync wait commands").  Tile's sem-assignment freely attaches
    several.  Move excess waits onto same-engine nops inserted right before
    the offending instruction (queue order makes that equivalent)."""
    for f in nc.m.functions:
        for bb in f.blocks:
            insts = bb.instructions
            insertions = []
            for idx, inst in enumerate(insts):
                si = inst.sync_info
                if si is None:
                    continue
                cap = 2 if isinstance(inst, mybir.InstEventSemaphore) else limit
                waits = list(si.on_wait)
                if len(waits) <= cap:
                    continue
                keep = waits[:cap]
                excess = waits[cap:]
                si.on_wait = keep
                nops = []
                for w in excess:
                    nop = mybir.InstNoOp(
                        name=nc.get_next_instruction_name(), ins=[], outs=[]
                    )
                    nop.engine = inst.engine
                    nop.sync_info = mybir.SyncInfo(on_wait=[w], on_update=[])
                    nc.register_instruction(nop, overwrite=True)
                    nops.append(nop)
                insertions.append((idx, nops))
            for idx, nops in reversed(insertions):
                for nop in reversed(nops):
                    bb.instructions.insert(idx, nop)


class PatchedTileContext(tile.TileContext):
    """Work around the 1-sync-wait-per-instruction cap in this walrus build:
    the stock TileContext tail drain carries one wait per outstanding proc,
    which codegen rejects ("Too many sync wait commands").  Split the waits
    across single-wait SP nops instead."""

    def _drain_and_barrier(self, tick_clock, wait_clock):
        drain_inst = self.nc.sync.drain()
        wait_clock.add_sem_waits(
            drain_inst.ins, ScopedClock({None: tick_clock.global_clock})
        )
        si = drain_inst.ins.sync_info
        if si is not None and len(si.on_wait) > 1:
            waits = list(si.on_wait)
            si.on_wait = [waits[0]]
            for w in waits[1:]:
                nop = self.nc.sync.nop(nofuse=True, hint="drain_wait_split")
                nop.ins.sync_info = mybir.SyncInfo(on_wait=[w], on_update=[])
        self.nc.all_engine_barrier()
        assert self.sems is not None
        popped = self.nc._tile_sem_poison_stack.pop()
        assert popped is self._sem_poison
        self.nc.clear_and_free_semaphores(list(self.sems.allocated().values()))
        self.nc.all_engine_barrier()


def _build_program(repeat: int = 1, *, ldw_skip: bool = False,
                   dma_only: bool = False, mm_only: bool = False,
                   decouple: bool = False, tcp_bufs: int = 6) -> bass.Bass:
    nc = bass.Bass()

    # moving operands, pre-transposed + fp8-quantized host-side as their SBUF
    # image [partition, k-chunk, free]; values carry a x16 scale
    featT = nc.dram_tensor("featT", [128, D // 128, QS], FP8, kind="ExternalInput")
    logitT = nc.dram_tensor("logitT", [128, C // 128, QS], FP8, kind="ExternalInput")
    maskh = nc.dram_tensor("maskh", [N, QS], FP8, kind="ExternalInput")  # mask/32
    # stationary operands (unscaled fp8)
    neT = nc.dram_tensor("neT", [128, D // 128, N], FP8, kind="ExternalInput")
    oeT = nc.dram_tensor("oeT", [128, D // 128, N], FP8, kind="ExternalInput")
    nlT = nc.dram_tensor("nlT", [128, C // 128, N], FP8, kind="ExternalInput")
    stats = nc.dram_tensor("stats", [N, 128], F32, kind="ExternalOutput")

    AX = mybir.AxisListType
    OP = mybir.AluOpType
    ACT = mybir.ActivationFunctionType
    DR = mybir.MatmulPerfMode.DoubleRow
    CB2 = C // 256  # 32 K=256 double-chunks (logit path)
    DB2 = D // 256  # 2 double-chunks (feat path)

    with PatchedTileContext(nc) as tc:
        with (
            tc.tile_pool(name="const", bufs=1) as const,
            tc.tile_pool(name="small", bufs=4) as small,
            tc.tile_pool(name="scr", bufs=2) as scrp,
            tc.tile_pool(name="ftp", bufs=2) as ftp,
        ):
            neT_sb = const.tile([128, D // 128, N], FP8)
            nc.sync.dma_start(out=neT_sb, in_=neT[:, :, :])
            oeT_sb = const.tile([128, D // 128, N], FP8)
            nc.sync.dma_start(out=oeT_sb, in_=oeT[:, :, :])
            nlT_sb = const.tile([128, C // 128, N], FP8)
            nc.sync.dma_start(out=nlT_sb, in_=nlT[:, :, :])

            mw_sb = const.tile([N, QS], F32)
            mh_sb = const.tile([N, QS], FP8)
            out_sb = const.tile([N, 128], F32)
            s1c_sb = const.tile([N, A_CHUNKS, 512], F32)  # scores1 parked in SBUF
            if dma_only or mm_only or decouple:
                nc.any.memset(out_sb, 0.0)

            # mm_only probe: single hoisted moving tiles, DMA'd once; the
            # per-iteration loop then runs pure TensorE work (garbage values)
            if mm_only or decouple:
                ft0_sb = const.tile([128, D // 128, QS], FP8)
                nc.sync.dma_start(out=ft0_sb, in_=featT[:, :, :])
                tcb0_sb = const.tile([128, 2, QS], FP8)
                nc.sync.dma_start(out=tcb0_sb, in_=logitT[:, 0:2, :])

            def stats_block(src, col_m, col_z, col_a, nch, mw3):
                """Per-row stats over a [128, nch, 512] block `src` holding
                MSCALE*scores: chunk maxes (scaled) -> cols [col_m, col_m+nch),
                exp-sums vs chunk max -> cols [col_z, ...), one total
                masked-weighted raw sum -> col_a.  mw3 is the matching
                [128, nch, 512] maskw view (true scale)."""
                nc.vector.tensor_reduce(
                    out=out_sb[:, col_m : col_m + nch], in_=src,
                    axis=AX.X, op=OP.max,
                )
                negm = small.tile([128, 8], F32, tag="negm")
                nc.vector.tensor_scalar_mul(
                    out=negm[:, :nch], in0=out_sb[:, col_m : col_m + nch],
                    scalar1=-INV,
                )
                for k in range(nch):
                    escr = scrp.tile([128, 512], F32, tag="escr")
                    nc.scalar.activation(
                        out=escr, in_=src[:, k, :], func=ACT.Exp,
                        bias=negm[:, k : k + 1], scale=INV,
                        accum_out=out_sb[:, col_z + k : col_z + k + 1],
                    )
                # a partial = sum maskw * s_raw (product scratch: s1c_sb)
                nc.vector.scalar_tensor_tensor(
                    out=s1c_sb[:, :nch, :], in0=src, scalar=INV, in1=mw3,
                    op0=OP.mult, op1=OP.mult,
                    accum_out=out_sb[:, col_a : col_a + 1],
                )

            loop_cm = tc.For_i(0, repeat, 1) if repeat > 1 else nullcontext()
            with loop_cm:
                if mm_only or decouple:
                    ft = ft0_sb
                if mm_only:
                    pass
                else:
                    nc.sync.dma_start(out=mh_sb, in_=maskh[:, :])
                    ft_dma = ftp.tile([128, D // 128, QS], FP8, tag="ft")
                    nc.scalar.dma_start(out=ft_dma, in_=featT[:, :, :])
                    if not decouple:
                        ft = ft_dma

                # ---- Phase A: feat path (fp8 DoubleRow, K=256 per pass).
                # PSUM results are parked in SBUF immediately (scores1) or
                # consumed by one DVE op (weight -> maskw) so the banks free
                # fast; the ACT-side stats run later, overlapped with phase B's
                # DMA stream, keeping the two HWDGE rings unobstructed.
                with tc.tile_pool(name="psum_a", bufs=1, space="PSUM") as psum_a:
                    for qh in range(2):
                        ps1 = psum_a.tile([128, 4, 512], F32, tag="ps1")
                        psw = psum_a.tile([128, 4, 512], F32, tag="psw")
                        for dc2 in range(DB2):
                            lhs_ne = neT_sb[:, 2 * dc2 : 2 * dc2 + 2, :]
                            lhs_oe = oeT_sb[:, 2 * dc2 : 2 * dc2 + 2, :]
                            # same-stationary matmuls run consecutively so the
                            # PE weight load can be skipped on all but the first
                            for ps, lhs in ((ps1, lhs_ne), (psw, lhs_oe)):
                                for qw in range(4):
                                    qs = slice(
                                        qh * 2048 + qw * 512,
                                        qh * 2048 + (qw + 1) * 512,
                                    )
                                    rhs = ft[:, 2 * dc2 : 2 * dc2 + 2, qs]
                                    if dma_only:
                                        continue
                                    mm = nc.tensor.matmul(
                                        ps[:, qw, :], lhs, rhs,
                                        start=(dc2 == 0), stop=(dc2 == DB2 - 1),
                                        perf_mode=DR,
                                    )
                                    if ldw_skip and qw > 0:
                                        mm.ins.ldweights = False
                        hs = slice(qh * 2048, (qh + 1) * 2048)
                        if dma_only or mm_only or decouple:
                            continue
                        # maskw = (16w + 16) * mask/32 = (w + 1) * 0.5 * mask
                        nc.vector.scalar_tensor_tensor(
                            out=mw_sb[:, hs].rearrange("p (c q) -> p c q", c=4),
                            in0=psw, scalar=MSCALE,
                            in1=mh_sb[:, hs].rearrange("p (c q) -> p c q", c=4),
                            op0=OP.add, op1=OP.mult,
                        )
                        # park scores1 (scaled) in SBUF so the banks free fast
                        nc.vector.tensor_copy(
                            out=s1c_sb[:, qh * 4 : (qh + 1) * 4, :], in_=ps1
                        )

                # ---- Phase B: logit path (fp8 DoubleRow, 8 psum accumulators)
                with (
                    tc.tile_pool(name="tcp", bufs=tcp_bufs) as tcp,
                    tc.tile_pool(name="psum_b", bufs=1, space="PSUM") as psum_b,
                ):
                    psB = psum_b.tile([128, 8, 512], F32, tag="psB")
                    mw3 = mw_sb.rearrange("p (c q) -> p c q", c=8)

                    def b_step(cb2):
                        if mm_only:
                            tcb = tcb0_sb
                        else:
                            tcb = tcp.tile([128, 2, QS], FP8, tag="tcb", name="tcb")
                            dma_eng = nc.sync if cb2 % 2 == 0 else nc.scalar
                            dma_eng.dma_start(
                                out=tcb, in_=logitT[:, 2 * cb2 : 2 * cb2 + 2, :]
                            )
                            if decouple:
                                tcb = tcb0_sb
                        if dma_only:
                            return
                        lhs = nlT_sb[:, 2 * cb2 : 2 * cb2 + 2, :]
                        for qw in range(8):
                            mm = nc.tensor.matmul(
                                psB[:, qw, :], lhs,
                                tcb[:, :, qw * 512 : (qw + 1) * 512],
                                start=(cb2 == 0), stop=(cb2 == CB2 - 1),
                                perf_mode=DR,
                            )
                            if ldw_skip and qw > 0:
                                mm.ins.ldweights = False

                    for cb2 in range(12):
                        b_step(cb2)
                    if not (dma_only or mm_only or decouple):
                        # deferred phase A stats, interleaved mid-stream so the
                        # DVE/ACT work hides under the logit DMA stream
                        nc.vector.tensor_reduce(
                            out=out_sb[:, 24:32], in_=mw3, axis=AX.X, op=OP.add,
                        )
                        stats_block(s1c_sb, 0, 8, 16, A_CHUNKS, mw3)
                    for cb2 in range(12, CB2):
                        b_step(cb2)
                    if not (dma_only or mm_only or decouple):
                        # phase B stats
                        stats_block(psB, 32, 64, 96, 8, mw3)

            nc.sync.dma_start(out=stats[:, :], in_=out_sb)

    _split_excess_waits(nc)
    return nc


_PROGRAM: bass.Bass | None = None
LAST_RESULTS = None  # BassKernelResults of the most recent run (for profiling)


def _get_program() -> bass.Bass:
    global _PROGRAM
    if _PROGRAM is None:
        _PROGRAM = _build_program()
    return _PROGRAM


def _fp8_bytes(a: np.ndarray, scale: float):
    """f32 array -> (torch uint8 tensor | numpy uint8) of e4m3 bytes of
    scale*a.  TRN fp8e4 (IEEE-ish, max 240) and torch/OCP e4m3fn encode
    identically for |x| < 240, which host pre-scaling guarantees."""
    try:
        import torch

        t = torch.from_numpy(np.ascontiguousarray(a))
        if scale != 1.0:
            t = t * scale
        return t.to(torch.float8_e4m3fn).view(torch.uint8)
    except ImportError:
        import ml_dtypes

        x = a * scale if scale != 1.0 else a
        return x.astype(ml_dtypes.float8_e4m3).view(np.uint8)


def _fp8_img(bytes_rk, r0, r1):
    """[R, K] e4m3 byte matrix rows r0:r1 -> [128, K//128, r1-r0] SBUF-image
    numpy array (ml_dtypes.float8_e4m3): img[p, c, r] = in[r0+r, c*128+p]."""
    import ml_dtypes

    R = r1 - r0
    if isinstance(bytes_rk, np.ndarray):
        k = bytes_rk.shape[1]
        img = np.ascontiguousarray(
            bytes_rk[r0:r1].reshape(R, k // 128, 128).transpose(2, 1, 0)
        )
        return img.view(ml_dtypes.float8_e4m3)
    k = bytes_rk.shape[1]
    img = bytes_rk[r0:r1].view(R, k // 128, 128).permute(2, 1, 0).contiguous()
    return img.numpy().view(ml_dtypes.float8_e4m3)


def host_prep(old_embeds, old_logits, new_embeds, new_logits, labels,
              feat_queue, logit_queue, queue_labels, header):
    """Scatter + normalize + mask + fp8 quantize + pre-transpose on host;
    returns per-core in_maps and the per-row positive counts M."""
    import ml_dtypes

    old_embeds = np.asarray(old_embeds, dtype=np.float32)
    old_logits = np.asarray(old_logits, dtype=np.float32)
    new_embeds = np.asarray(new_embeds, dtype=np.float32)
    new_logits = np.asarray(new_logits, dtype=np.float32)
    feat_queue = np.array(feat_queue, dtype=np.float32)   # copies (scattered below)
    logit_queue = np.array(logit_queue, dtype=np.float32)
    labels_np = np.asarray(labels).astype(np.int64)
    queue_labels_np = np.asarray(queue_labels).astype(np.int64)
    hdr = int(np.asarray(header))

    n = old_embeds.shape[0]
    q = feat_queue.shape[0]
    assert (n, q) == (N, Q)

    # circular queue scatter
    idx = (hdr + np.arange(n)) % q
    feat_queue[idx] = old_embeds
    logit_queue[idx] = old_logits
    queue_labels_np[idx] = labels_np

    # normalize new_embeds (f64 intermediate, f32 result)
    ne64 = new_embeds.astype(np.float64)
    norm = np.sqrt((ne64 * ne64).sum(axis=1, keepdims=True))
    new_e = (ne64 / np.maximum(norm, EPS)).astype(np.float32)

    # label mask (host): maskh = mask/32 (exact in e4m3)
    mask = (queue_labels_np[None, :] == labels_np[:, None])
    M = mask.sum(axis=1).astype(np.float64)               # [N], >= 1 by construction
    maskh = (mask.astype(np.float32) / 32.0).astype(ml_dtypes.float8_e4m3)

    fq8 = _fp8_bytes(feat_queue, MSCALE)       # [Q, D] bytes, x16
    lq8 = _fp8_bytes(logit_queue, MSCALE)      # [Q, C] bytes, x16
    neT = _fp8_img(_fp8_bytes(new_e, 1.0), 0, N)          # [128, 4, N]
    oeT = _fp8_img(_fp8_bytes(old_embeds, 1.0), 0, N)     # [128, 4, N]
    nlT = _fp8_img(_fp8_bytes(new_logits, 1.0), 0, N)     # [128, 64, N]

    in_maps = []
    for d in range(N_CORES):
        in_maps.append({
            "featT": _fp8_img(fq8, d * QS, (d + 1) * QS),     # [128, 4, QS]
            "logitT": _fp8_img(lq8, d * QS, (d + 1) * QS),    # [128, 64, QS]
            "maskh": np.ascontiguousarray(maskh[:, d * QS : (d + 1) * QS]),
            "neT": neT,
            "oeT": oeT,
            "nlT": nlT,
        })
    return in_maps, M


def combine_stats(parts: np.ndarray, M: np.ndarray):
    """parts: [n_cores, 128, 128] f32 stats tiles -> (l1, l2) f32 scalars."""
    parts = parts.astype(np.float64)
    m1p = parts[:, :, 0:8] / MSCALE            # chunk maxes carry the x16 scale
    z1p = parts[:, :, 8:16]
    a1p = parts[:, :, 16:17]
    wp = parts[:, :, 24:32]
    m2p = parts[:, :, 32 : 32 + B_CHUNKS] / MSCALE
    z2p = parts[:, :, 64 : 64 + B_CHUNKS]
    a2p = parts[:, :, 96:97]

    W = wp.sum(axis=(0, 2))                               # [N]
    A1 = a1p.sum(axis=(0, 2))
    A2 = a2p.sum(axis=(0, 2))
    m1 = m1p.max(axis=(0, 2))
    m2 = m2p.max(axis=(0, 2))
    Z1 = (z1p * np.exp(m1p - m1[None, :, None])).sum(axis=(0, 2))
    Z2 = (z2p * np.exp(m2p - m2[None, :, None])).sum(axis=(0, 2))

    # sum_j maskw * log_prob = A_raw - (m + log Z) * W ; divide by count, mean, negate
    l1 = -np.mean((A1 - (m1 + np.log(Z1)) * W) / M)
    l2 = -np.mean((A2 - (m2 + np.log(Z2)) * W) / M)
    return (np.float32(l1), np.float32(l2))


def kernel(old_embeds, old_logits, new_embeds, new_logits, labels,
           feat_queue, logit_queue, queue_labels, header):
    global LAST_RESULTS
    in_maps, M = host_prep(
        old_embeds, old_logits, new_embeds, new_logits, labels,
        feat_queue, logit_queue, queue_labels, header,
    )
    nc = _get_program()
    LAST_RESULTS = run_bass_kernel_spmd(nc, in_maps, list(range(N_CORES)))
    parts = np.stack([LAST_RESULTS.results[d]["stats"] for d in range(N_CORES)])
    return combine_stats(parts, M)


# revision 19
# speedup vs baseline: 1.0043x; 1.0043x over previous
"""Trainium2 Bass kernel for nn_CompatibleLearningLoss (MoCo-style queue contrastive loss).

Strategy: shard the queue dimension (Q=32768) across 8 NeuronCores (4096 rows
each).  Each core computes its slice of the three matmuls
    old_embeds  @ feat_queue_shard.T   -> weight
    new_e       @ feat_queue_shard.T   -> scores1
    new_logits  @ logit_queue_shard.T  -> scores2
and reduces per-row partial softmax statistics (chunk max, exp-sum vs chunk
max, masked-weighted raw sums) into a [128, 128] stats tile.  The host combines
the partials in float64 and produces the two scalar losses.

The kernel is HBM-bandwidth bound (the logit-queue stream dominates), so both
queue matrices travel as fp8e4 (TRN E4M3) and every matmul runs in
perf_mode=DoubleRow (K=256 per pass).  The moving operands are pre-scaled by
16 host-side to clear the e4m3 subnormal band; the 1/16 descale is folded into
the Exp activation / the maskw fuse / the host-side combine.  Measured loss
error ~3e-3 relative (gate is 2e-2).

Marshaling (all host-side): circular queue scatter, new_embeds normalization,
label mask, fp8 quantization, and pre-transposition of everything into its
SBUF image [partition, k-chunk, free] so each DMA is one contiguous run per
partition.
"""

from contextlib import nullcontext

import numpy as np

import concourse.bass as bass
import concourse.tile as tile
from concourse import mybir
from concourse.bass_utils import run_bass_kernel_spmd
from concourse.vector_clock import ScopedClock

N = 128      # batch
D = 512      # embed dim
C = 8192     # logit dim
Q = 32768    # queue length
N_CORES = 8
QS = Q // N_CORES          # 4096 queue rows per core
EPS = 1e-12
MSCALE = 16.0              # host pre-scale on the moving (queue) operands
INV = 1.0 / MSCALE

F32 = mybir.dt.float32
BF16 = mybir.dt.bfloat16
FP8 = mybir.dt.float8e4

# stats tile column layout (per core, [128, 128] f32)
# cols 0:8      m1 parts   (feat-path chunk maxes TIMES MSCALE, 8 chunks of 512)
# cols 8:16     z1 parts   (feat-path sum exp(s - chunk max))
# cols 16:24    a1 parts   (feat-path sum maskw * s_raw)
# cols 24:32    W  parts   (sum maskw)
# cols 32:32+B  m2 parts   (logit-path chunk maxes TIMES MSCALE, B_CHUNKS chunks of 512)
# cols 64:64+B  z2 parts
# cols 96:96+B  a2 parts
A_CHUNKS = 8
B_CHUNKS = 8


def _split_excess_waits(nc: bass.Bass, limit: int = 1) -> None:
    """This walrus build rejects instructions carrying more than one sync wait
    ("Too many sync wait commands").  Tile's sem-assignment freely attaches
    several.  Move excess waits onto same-engine nops inserted right before
    the offending instruction (queue order makes that equivalent)."""
    for f in nc.m.functions:
        for bb in f.blocks:
            insts = bb.instructions
            insertions = []
            for idx, inst in enumerate(insts):
                si = inst.sync_info
                if si is None:
                    continue
                cap = 2 if isinstance(inst, mybir.InstEventSemaphore) else limit
                waits = list(si.on_wait)
                if len(waits) <= cap:
                    continue
                keep = waits[:cap]
                excess = waits[cap:]
                si.on_wait = keep
                nops = []
                for w in excess:
                    nop = mybir.InstNoOp(
                        name=nc.get_next_instruction_name(), ins=[], outs=[]
                    )
                    nop.engine = inst.engine
                    nop.sync_info = mybir.SyncInfo(on_wait=[w], on_update=[])
                    nc.register_instruction(nop, overwrite=True)
                    nops.append(nop)
                insertions.append((idx, nops))
            for idx, nops in reversed(insertions):
                for nop in reversed(nops):
                    bb.instructions.insert(idx, nop)


class PatchedTileContext(tile.TileContext):
    """Work around the 1-sync-wait-per-instruction cap in this walrus build:
    the stock TileContext tail drain carries one wait per outstanding proc,
    which codegen rejects ("Too many sync wait commands").  Split the waits
    across single-wait SP nops instead."""

    def _drain_and_barrier(self, tick_clock, wait_clock):
        drain_inst = self.nc.sync.drain()
        wait_clock.add_sem_waits(
            drain_inst.ins, ScopedClock({None: tick_clock.global_clock})
        )
        si = drain_inst.ins.sync_info
        if si is not None and len(si.on_wait) > 1:
            waits = list(si.on_wait)
            si.on_wait = [waits[0]]
            for w in waits[1:]:
                nop = self.nc.sync.nop(nofuse=True, hint="drain_wait_split")
                nop.ins.sync_info = mybir.SyncInfo(on_wait=[w], on_update=[])
        self.nc.all_engine_barrier()
        assert self.sems is not None
        popped = self.nc._tile_sem_poison_stack.pop()
        assert popped is self._sem_poison
        self.nc.clear_and_free_semaphores(list(self.sems.allocated().values()))
        self.nc.all_engine_barrier()


def _build_program(repeat: int = 1, *, ldw_skip: bool = False,
                   dma_only: bool = False, mm_only: bool = False,
                   decouple: bool = False, tcp_bufs: int = 6,
                   top_q: str = "scalar", n_chunk_q: int = 2,
                   ldw_pair: bool = False, big_chunks: bool = False) -> bass.Bass:
    nc = bass.Bass()

    # moving operands, pre-transposed + fp8-quantized host-side as their SBUF
    # image [partition, k-chunk, free]; values carry a x16 scale
    featT = nc.dram_tensor("featT", [128, D // 128, QS], FP8, kind="ExternalInput")
    logitT = nc.dram_tensor("logitT", [128, C // 128, QS], FP8, kind="ExternalInput")
    maskh = nc.dram_tensor("maskh", [N, QS], FP8, kind="ExternalInput")  # mask/32
    # stationary operands (unscaled fp8)
    neT = nc.dram_tensor("neT", [128, D // 128, N], FP8, kind="ExternalInput")
    oeT = nc.dram_tensor("oeT", [128, D // 128, N], FP8, kind="ExternalInput")
    nlT = nc.dram_tensor("nlT", [128, C // 128, N], FP8, kind="ExternalInput")
    stats = nc.dram_tensor("stats", [N, 128], F32, kind="ExternalOutput")

    AX = mybir.AxisListType
    OP = mybir.AluOpType
    ACT = mybir.ActivationFunctionType
    DR = mybir.MatmulPerfMode.DoubleRow
    CB2 = C // 256  # 32 K=256 double-chunks (logit path)
    DB2 = D // 256  # 2 double-chunks (feat path)

    with PatchedTileContext(nc) as tc:
        with (
            tc.tile_pool(name="const", bufs=1) as const,
            tc.tile_pool(name="small", bufs=4) as small,
            tc.tile_pool(name="scr", bufs=2) as scrp,
            tc.tile_pool(name="ftp", bufs=2) as ftp,
        ):
            neT_sb = const.tile([128, D // 128, N], FP8)
            nc.sync.dma_start(out=neT_sb, in_=neT[:, :, :])
            oeT_sb = const.tile([128, D // 128, N], FP8)
            nc.sync.dma_start(out=oeT_sb, in_=oeT[:, :, :])
            nlT_sb = const.tile([128, C // 128, N], FP8)
            nc.sync.dma_start(out=nlT_sb, in_=nlT[:, :, :])

            mw_sb = const.tile([N, QS], BF16)
            mh_sb = const.tile([N, QS], FP8)
            out_sb = const.tile([N, 128], F32)
            s1c_sb = const.tile([N, A_CHUNKS, 512], BF16)  # scores1 parked in SBUF
            if dma_only or mm_only or decouple:
                nc.any.memset(out_sb, 0.0)

            # mm_only probe: single hoisted moving tiles, DMA'd once; the
            # per-iteration loop then runs pure TensorE work (garbage values)
            if mm_only or decouple:
                ft0_sb = const.tile([128, D // 128, QS], FP8)
                nc.sync.dma_start(out=ft0_sb, in_=featT[:, :, :])
                tcb0_sb = const.tile([128, 2, QS], FP8)
                nc.sync.dma_start(out=tcb0_sb, in_=logitT[:, 0:2, :])

            def stats_block(src, col_m, col_z, col_a, nch, mw3):
                """Per-row stats over a [128, nch, 512] block `src` holding
                MSCALE*scores: chunk maxes (scaled) -> cols [col_m, col_m+nch),
                exp-sums vs chunk max -> cols [col_z, ...), one total
                masked-weighted raw sum -> col_a.  mw3 is the matching
                [128, nch, 512] maskw view (true scale)."""
                nc.vector.tensor_reduce(
                    out=out_sb[:, col_m : col_m + nch], in_=src,
                    axis=AX.X, op=OP.max,
                )
                negm = small.tile([128, 8], F32, tag="negm")
                nc.vector.tensor_scalar_mul(
                    out=negm[:, :nch], in0=out_sb[:, col_m : col_m + nch],
                    scalar1=-INV,
                )
                for k in range(nch):
                    escr = scrp.tile([128, 512], BF16, tag="escr")
                    nc.scalar.activation(
                        out=escr, in_=src[:, k, :], func=ACT.Exp,
                        bias=negm[:, k : k + 1], scale=INV,
                        accum_out=out_sb[:, col_z + k : col_z + k + 1],
                    )
                # a partial = sum maskw * s_raw (product scratch: s1c_sb)
                nc.vector.scalar_tensor_tensor(
                    out=s1c_sb[:, :nch, :], in0=src, scalar=INV, in1=mw3,
                    op0=OP.mult, op1=OP.mult,
                    accum_out=out_sb[:, col_a : col_a + 1],
                )

            loop_cm = tc.For_i(0, repeat, 1) if repeat > 1 else nullcontext()
            with loop_cm:
                if mm_only or decouple:
                    ft = ft0_sb
                if mm_only:
                    pass
                else:
                    nc.sync.dma_start(out=mh_sb, in_=maskh[:, :])
                    ft_dma = ftp.tile([128, D // 128, QS], FP8, tag="ft")
                    getattr(nc, top_q).dma_start(out=ft_dma, in_=featT[:, :, :])
                    if not decouple:
                        ft = ft_dma

                # ---- Phase A: feat path (fp8 DoubleRow, K=256 per pass).
                # PSUM results are parked in SBUF immediately (scores1) or
                # consumed by one DVE op (weight -> maskw) so the banks free
                # fast; the ACT-side stats run later, overlapped with phase B's
                # DMA stream, keeping the two HWDGE rings unobstructed.
                with tc.tile_pool(name="psum_a", bufs=1, space="PSUM") as psum_a:
                    for qh in range(2):
                        ps1 = psum_a.tile([128, 4, 512], F32, tag="ps1")
                        psw = psum_a.tile([128, 4, 512], F32, tag="psw")
                        for dc2 in range(DB2):
                            lhs_ne = neT_sb[:, 2 * dc2 : 2 * dc2 + 2, :]
                            lhs_oe = oeT_sb[:, 2 * dc2 : 2 * dc2 + 2, :]
                            # same-stationary matmuls run consecutively so the
                            # PE weight load can be skipped on all but the first
                            for ps, lhs in ((ps1, lhs_ne), (psw, lhs_oe)):
                                for qw in range(4):
                                    qs = slice(
                                        qh * 2048 + qw * 512,
                                        qh * 2048 + (qw + 1) * 512,
                                    )
                                    rhs = ft[:, 2 * dc2 : 2 * dc2 + 2, qs]
                                    if dma_only:
                                        continue
                                    mm = nc.tensor.matmul(
                                        ps[:, qw, :], lhs, rhs,
                                        start=(dc2 == 0), stop=(dc2 == DB2 - 1),
                                        perf_mode=DR,
                                    )
                                    if ldw_skip and qw > 0:
                                        mm.ins.ldweights = False
                        hs = slice(qh * 2048, (qh + 1) * 2048)
                        if dma_only or mm_only or decouple:
                            continue
                        # maskw = (16w + 16) * mask/32 = (w + 1) * 0.5 * mask
                        nc.vector.scalar_tensor_tensor(
                            out=mw_sb[:, hs].rearrange("p (c q) -> p c q", c=4),
                            in0=psw, scalar=MSCALE,
                            in1=mh_sb[:, hs].rearrange("p (c q) -> p c q", c=4),
                            op0=OP.add, op1=OP.mult,
                        )
                        # park scores1 (scaled) in SBUF so the banks free fast
                        nc.scalar.activation(
                            out=s1c_sb[:, qh * 4 : (qh + 1) * 4, :], in_=ps1,
                            func=ACT.Copy,
                        )

                # ---- Phase B: logit path (fp8 DoubleRow, 8 psum accumulators)
                with (
                    tc.tile_pool(name="tcp", bufs=tcp_bufs) as tcp,
                    tc.tile_pool(name="psum_b", bufs=1, space="PSUM") as psum_b,
                ):
                    psB = psum_b.tile([128, 8, 512], F32, tag="psB")
                    mw3 = mw_sb.rearrange("p (c q) -> p c q", c=8)

                    CPC = 2 if big_chunks else 1   # K-double-chunks per DMA
                    NG = CB2 // CPC                # DMA groups

                    def b_step(g):
                        if mm_only:
                            tcb = tcb0_sb
                        else:
                            tcb = tcp.tile([128, 2 * CPC, QS], FP8, tag="tcb",
                                           name="tcb")
                            dma_eng = (nc.sync if (g % 2 == 0 or g == NG - 1)
                                       else nc.scalar)
                            dma_eng.dma_start(
                                out=tcb,
                                in_=logitT[:, 2 * CPC * g : 2 * CPC * (g + 1), :],
                            )
                            if decouple:
                                tcb = tcb0_sb
                        if dma_only:
                            return
                        for j in range(CPC):
                            cb2 = CPC * g + j
                            lhs = nlT_sb[:, 2 * cb2 : 2 * cb2 + 2, :]
                            if ldw_pair:
                                nc.tensor.ldweights(lhs, perf_mode=DR)
                            for qw in range(8):
                                mm = nc.tensor.matmul(
                                    psB[:, qw, :], lhs,
                                    tcb[:, 2 * j : 2 * j + 2,
                                        qw * 512 : (qw + 1) * 512],
                                    start=(cb2 == 0), stop=(cb2 == CB2 - 1),
                                    perf_mode=DR,
                                )
                                if ldw_pair or (ldw_skip and qw > 0):
                                    mm.ins.ldweights = False

                    for g in range(12 // CPC):
                        b_step(g)
                    if not (dma_only or mm_only or decouple):
                        # deferred phase A stats, interleaved mid-stream so the
                        # DVE/ACT work hides under the logit DMA stream
                        nc.vector.tensor_reduce(
                            out=out_sb[:, 24:32], in_=mw3, axis=AX.X, op=OP.add,
                        )
                        stats_block(s1c_sb, 0, 8, 16, A_CHUNKS, mw3)
                    for g in range(12 // CPC, NG):
                        b_step(g)
                    if not (dma_only or mm_only or decouple):
                        # phase B stats
                        stats_block(psB, 32, 64, 96, 8, mw3)

            nc.sync.dma_start(out=stats[:, :], in_=out_sb)

    _split_excess_waits(nc)
    return nc


_PROGRAM: bass.Bass | None = None
LAST_RESULTS = None  # BassKernelResults of the most recent run (for profiling)


def _get_program() -> bass.Bass:
    global _PROGRAM
    if _PROGRAM is None:
        _PROGRAM = _build_program()
    return _PROGRAM


def _fp8_bytes(a: np.ndarray, scale: float):
    """f32 array -> (torch uint8 tensor | numpy uint8) of e4m3 bytes of
    scale*a.  TRN fp8e4 (IEEE-ish, max 240) and torch/OCP e4m3fn encode
    identically for |x| < 240, which host pre-scaling guarantees."""
    try:
        import torch

        t = torch.from_numpy(np.ascontiguousarray(a))
        if scale != 1.0:
            t = t * scale
        return t.to(torch.float8_e4m3fn).view(torch.uint8)
    except ImportError:
        import ml_dtypes

        x = a * scale if scale != 1.0 else a
        return x.astype(ml_dtypes.float8_e4m3).view(np.uint8)


def _fp8_img(bytes_rk, r0, r1):
    """[R, K] e4m3 byte matrix rows r0:r1 -> [128, K//128, r1-r0] SBUF-image
    numpy array (ml_dtypes.float8_e4m3): img[p, c, r] = in[r0+r, c*128+p]."""
    import ml_dtypes

    R = r1 - r0
    if isinstance(bytes_rk, np.ndarray):
        k = bytes_rk.shape[1]
        img = np.ascontiguousarray(
            bytes_rk[r0:r1].reshape(R, k // 128, 128).transpose(2, 1, 0)
        )
        return img.view(ml_dtypes.float8_e4m3)
    k = bytes_rk.shape[1]
    img = bytes_rk[r0:r1].view(R, k // 128, 128).permute(2, 1, 0).contiguous()
    return img.numpy().view(ml_dtypes.float8_e4m3)


def host_prep(old_embeds, old_logits, new_embeds, new_logits, labels,
              feat_queue, logit_queue, queue_labels, header):
    """Scatter + normalize + mask + fp8 quantize + pre-transpose on host;
    returns per-core in_maps and the per-row positive counts M."""
    import ml_dtypes

    old_embeds = np.asarray(old_embeds, dtype=np.float32)
    old_logits = np.asarray(old_logits, dtype=np.float32)
    new_embeds = np.asarray(new_embeds, dtype=np.float32)
    new_logits = np.asarray(new_logits, dtype=np.float32)
    feat_queue = np.array(feat_queue, dtype=np.float32)   # copies (scattered below)
    logit_queue = np.array(logit_queue, dtype=np.float32)
    labels_np = np.asarray(labels).astype(np.int64)
    queue_labels_np = np.asarray(queue_labels).astype(np.int64)
    hdr = int(np.asarray(header))

    n = old_embeds.shape[0]
    q = feat_queue.shape[0]
    assert (n, q) == (N, Q)

    # circular queue scatter
    idx = (hdr + np.arange(n)) % q
    feat_queue[idx] = old_embeds
    logit_queue[idx] = old_logits
    queue_labels_np[idx] = labels_np

    # normalize new_embeds (f64 intermediate, f32 result)
    ne64 = new_embeds.astype(np.float64)
    norm = np.sqrt((ne64 * ne64).sum(axis=1, keepdims=True))
    new_e = (ne64 / np.maximum(norm, EPS)).astype(np.float32)

    # label mask (host): maskh = mask/32 (exact in e4m3)
    mask = (queue_labels_np[None, :] == labels_np[:, None])
    M = mask.sum(axis=1).astype(np.float64)               # [N], >= 1 by construction
    maskh = (mask.astype(np.float32) / 32.0).astype(ml_dtypes.float8_e4m3)

    fq8 = _fp8_bytes(feat_queue, MSCALE)       # [Q, D] bytes, x16
    lq8 = _fp8_bytes(logit_queue, MSCALE)      # [Q, C] bytes, x16
    neT = _fp8_img(_fp8_bytes(new_e, 1.0), 0, N)          # [128, 4, N]
    oeT = _fp8_img(_fp8_bytes(old_embeds, 1.0), 0, N)     # [128, 4, N]
    nlT = _fp8_img(_fp8_bytes(new_logits, 1.0), 0, N)     # [128, 64, N]

    in_maps = []
    for d in range(N_CORES):
        in_maps.append({
            "featT": _fp8_img(fq8, d * QS, (d + 1) * QS),     # [128, 4, QS]
            "logitT": _fp8_img(lq8, d * QS, (d + 1) * QS),    # [128, 64, QS]
            "maskh": np.ascontiguousarray(maskh[:, d * QS : (d + 1) * QS]),
            "neT": neT,
            "oeT": oeT,
            "nlT": nlT,
        })
    return in_maps, M


def combine_stats(parts: np.ndarray, M: np.ndarray):
    """parts: [n_cores, 128, 128] f32 stats tiles -> (l1, l2) f32 scalars."""
    parts = parts.astype(np.float64)
    m1p = parts[:, :, 0:8] / MSCALE            # chunk maxes carry the x16 scale
    z1p = parts[:, :, 8:16]
    a1p = parts[:, :, 16:17]
    wp = parts[:, :, 24:32]
    m2p = parts[:, :, 32 : 32 + B_CHUNKS] / MSCALE
    z2p = parts[:, :, 64 : 64 + B_CHUNKS]
    a2p = parts[:, :, 96:97]

    W = wp.sum(axis=(0, 2))                               # [N]
    A1 = a1p.sum(axis=(0, 2))
    A2 = a2p.sum(axis=(0, 2))
    m1 = m1p.max(axis=(0, 2))
    m2 = m2p.max(axis=(0, 2))
    Z1 = (z1p * np.exp(m1p - m1[None, :, None])).sum(axis=(0, 2))
    Z2 = (z2p * np.exp(m2p - m2[None, :, None])).sum(axis=(0, 2))

    # sum_j maskw * log_prob = A_raw - (m + log Z) * W ; divide by count, mean, negate
    l1 = -np.mean((A1 - (m1 + np.log(Z1)) * W) / M)
    l2 = -np.mean((A2 - (m2 + np.log(Z2)) * W) / M)
    return (np.float32(l1), np.float32(l2))


def kernel(old_embeds, old_logits, new_embeds, new_logits, labels,
           feat_queue, logit_queue, queue_labels, header):
    global LAST_RESULTS
    in_maps, M = host_prep(
        old_embeds, old_logits, new_embeds, new_logits, labels,
        feat_queue, logit_queue, queue_labels, header,
    )
    nc = _get_program()
    LAST_RESULTS = run_bass_kernel_spmd(nc, in_maps, list(range(N_CORES)))
    parts = np.stack([LAST_RESULTS.results[d]["stats"] for d in range(N_CORES)])
    return combine_stats(parts, M)
